# revision 1
# baseline (speedup 1.0000x reference)
"""CrossTableAttention Trainium2 kernel (8-core SPMD, batch-sharded).

Math (per table t, row b):
  rel_w[t,r]   = sigmoid(rel_embs[t,r] . w_rel + b_rel)          (host, tiny)
  Qp[t]        = emb[t] @ Wq.T (+bq)                              [B, D]
  Kb[j]        = emb[j] @ Wk.T        (bias bk is softmax-invariant -> dropped)
  Vb[j]        = emb[j] @ Wv.T        (bias bv folded into output bias)
  score[t,b,h,r] = rel_w[t,r] * (Qp[t,b,h,:] . Kb[j_r,b,h,:]) / sqrt(DH)
  attn         = softmax_r(score);  P = attn * rel_w
  ctx[t,b]     = sum_r P[t,b,h,r] * Vb[j_r,b,h,:]
  out[t]       = ctx[t] @ Wo.T + (Wo @ bv + bo)

Key algebraic optimization: K/V projections are computed per *table* (16) instead
of per (table, relation) gather (128) - the relation weight is a scalar that
commutes with the linear projection.  5x fewer matmul FLOPs than the reference.

Device layout notes:
 - Activations live as [row(b) on partitions, feature on free] so the attention
   inner products reduce along the free axis (DVE).
 - q/k/v/ctx feature axis is permuted to dh-major (f = dh*16 + h) so the
   per-(b,h) attention-weight broadcast has a step-1 innermost AP dim
   (keeps DVE tensor_tensor in 2x bf16 mode).  Weight matrices are permuted
   host-side to produce/consume this layout directly.
 - scores/P layout per t: [128 b, 128 = r*16 + h] (h innermost).
 - Matmul operands need the contraction dim (d) on partitions; embT is produced
   by bf16 DMA-xbar transposing loads straight from the (host pre-cast) input.
"""

import sys

sys.path.insert(0, "/opt/trn_rl_repo")

import numpy as np
import ml_dtypes

import concourse.bass as bass
import concourse.bacc as bacc_mod
import concourse.mybir as mybir
import concourse.tile as tile
from concourse.bass_utils import run_bass_kernel_spmd

T, B, D, R, H = 16, 1024, 1024, 8, 16
DH = D // H  # 64
NCORES = 8
BC = B // NCORES  # 128 rows per core
KCH = D // 128  # 8 contraction chunks

F32 = mybir.dt.float32
BF16 = mybir.dt.bfloat16
AX = mybir.AxisListType
AF = mybir.ActivationFunctionType

# feature permutation: new f = dh*16 + h  <->  old o = h*64 + dh
_PERM = np.array([(f % H) * DH + f // H for f in range(D)], dtype=np.int64)


def _bcast_free(ap, n, pos):
    """Insert a [step=0, n] broadcast dim into an AP's free dims at `pos`
    (pos counts free dims, 0 = outermost free dim)."""
    new = list(ap.ap)
    new.insert(1 + pos, [0, n])
    return bass.AP(tensor=ap.tensor, offset=ap.offset, ap=new)


def _bcast_part(ap, n=128):
    """Broadcast a (DRAM) AP across n partitions by prepending a [0, n] dim."""
    return bass.AP(tensor=ap.tensor, offset=ap.offset, ap=[[0, n]] + list(ap.ap))


def _structure(rel_idx):
    """Host-side dedup of the (t, j) gather structure."""
    groups = []  # per t: list of (j, r0, [extra r's])
    for t in range(T):
        by_j = {}
        for r in range(R):
            by_j.setdefault(int(rel_idx[t, r]), []).append(r)
        groups.append([(j, rs[0], rs[1:]) for j, rs in sorted(by_j.items())])
    pairs_by_j = {j: [] for j in range(T)}
    for t in range(T):
        for j, r0, extras in groups[t]:
            pairs_by_j[j].append((t, r0))
    # first j (in ascending j processing order) touching each t
    first_j = {}
    for j in range(T):
        for t, _ in pairs_by_j[j]:
            if t not in first_j:
                first_j[t] = j
    # last j touching each t (to schedule ctx output DMA)
    last_j = {}
    for j in range(T):
        for t, _ in pairs_by_j[j]:
            last_j[t] = j
    return groups, pairs_by_j, first_j, last_j


def _build(rel_idx, use_bq, use_bo):
    """Build the SPMD bass program (identical on all cores).

    v2 structure (from v1 trace analysis):
     - Q+K projections interleaved per t (shared stationary embT chunk ->
       LDWEIGHTS amortized over 4 matmuls), Kb fully SBUF-resident so the
       scores phase never back-pressures the PE.
     - score pairs emitted in availability order (sorted by max(t, j)) so the
       DVE trails the projection stream without stalls.
     - per-head dot products via a binary tree of tensor_tensor adds (bf16 2x
       mode for the large levels, fp32 tail) instead of 1x tensor_reduce.
     - V projections recomputed in the ctx phase (keeps PE busy there);
       ctx accumulation adds alternate DVE / GPSIMD.
    """
    groups, pairs_by_j, first_j, last_j = _structure(rel_idx)

    # (t, j, r0, extras) sorted by when both Qp[t] and Kb[j] become available
    pair_sched = []
    for t in range(T):
        for j, r0, extras in groups[t]:
            pair_sched.append((max(t, j), t, j, r0, extras))
    pair_sched.sort(key=lambda x: (x[0], x[1]))
    last_pair_of_t = {}
    for i, (_, t, j, r0, ex) in enumerate(pair_sched):
        last_pair_of_t[t] = i

    nc = bacc_mod.Bacc(None, target_bir_lowering=False, debug=False)
    emb_ext = nc.dram_tensor("emb", [T, BC, D], BF16, kind="ExternalInput")
    wq_ext = nc.dram_tensor("wq", [D, D], BF16, kind="ExternalInput")
    wk_ext = nc.dram_tensor("wk", [D, D], BF16, kind="ExternalInput")
    wv_ext = nc.dram_tensor("wv", [D, D], BF16, kind="ExternalInput")
    wo_ext = nc.dram_tensor("wo", [D, D], BF16, kind="ExternalInput")
    rw_ext = nc.dram_tensor("rw", [T, R * H], F32, kind="ExternalInput")
    if use_bq:
        bq_ext = nc.dram_tensor("bqp", [D], F32, kind="ExternalInput")
    if use_bo:
        bo_ext = nc.dram_tensor("boe", [D], F32, kind="ExternalInput")
    out_ext = nc.dram_tensor("out", [T, BC, D], F32, kind="ExternalOutput")

    with tile.TileContext(nc) as tc:
        with (
            tc.tile_pool(name="consts", bufs=1) as consts,
            tc.tile_pool(name="wpool", bufs=2) as wpool,
            tc.tile_pool(name="big", bufs=1) as big,
            tc.tile_pool(name="embp", bufs=1) as embp,
            tc.tile_pool(name="kball", bufs=1) as kballp,
            tc.tile_pool(name="attn", bufs=1) as attnp,
            tc.tile_pool(name="kv", bufs=3) as kvp,
            tc.tile_pool(name="work", bufs=3) as work,
            tc.tile_pool(name="smalls", bufs=3) as smalls,
            tc.tile_pool(name="outp", bufs=2) as outp,
            tc.tile_pool(name="ctxT", bufs=2) as ctxTp,
            tc.tile_pool(name="psum", bufs=8, space="PSUM") as psum,
            tc.tile_pool(name="dram", bufs=1, space="DRAM") as dramp,
        ):
            ctx_dram = dramp.tile([T, BC, D], BF16)
            # ---- constant + weight loads ----
            rw_full = consts.tile([128, T, R * H], F32)
            nc.gpsimd.dma_start(out=rw_full, in_=_bcast_part(rw_ext[:]))
            if use_bq:
                bq_full = consts.tile([128, D], F32)
                nc.gpsimd.dma_start(out=bq_full, in_=_bcast_part(bq_ext[:]))
            if use_bo:
                bo_full = consts.tile([128, D], F32)
                nc.gpsimd.dma_start(out=bo_full, in_=_bcast_part(bo_ext[:]))

            wq_t = wpool.tile([128, KCH, D], BF16, tag="w")
            nc.gpsimd.dma_start(
                out=wq_t, in_=wq_ext.rearrange("(k p) o -> p k o", p=128)
            )
            wk_t = wpool.tile([128, KCH, D], BF16, tag="w")
            nc.gpsimd.dma_start(
                out=wk_t, in_=wk_ext.rearrange("(k p) o -> p k o", p=128)
            )

            # embT[p, k, t*BC+b] = emb[t, b, k*128+p]  (bf16 xbar transposes,
            # split across both HWDGE sequencers to shorten the startup gate)
            embT = embp.tile([128, KCH, T * BC], BF16)
            for k in range(KCH):
                nc.scalar.dma_start_transpose(
                    out=embT[:, k, :],
                    in_=emb_ext[:, :, k * 128 : (k + 1) * 128].rearrange(
                        "t b d -> (t b) d"
                    ),
                )

            qp_all = big.tile([128, T, D], BF16, tag="qpctx")
            kb_all = kballp.tile([128, T, D], BF16)
            scores = attnp.tile([128, T, R * H], F32)
            p2_all = attnp.tile([128, T, R * H], BF16)

            def emit_pair(t, j, r0, extras):
                """scores[t, r0-block] = per-head dot(Qp[t], Kb[j]) via
                bf16 mul + binary-tree halving over dh (dh-major layout:
                folding dh halves == folding contiguous column halves)."""
                prod = work.tile([128, D], BF16, tag="prod")
                nc.vector.tensor_mul(prod, qp_all[:, t, :], kb_all[:, j, :])
                nc.vector.tensor_add(prod[:, 0:512], prod[:, 0:512], prod[:, 512:1024])
                nc.vector.tensor_add(prod[:, 0:256], prod[:, 0:256], prod[:, 256:512])
                sc32 = smalls.tile([128, 128], F32, tag="sc32")
                nc.vector.tensor_add(sc32, prod[:, 0:128], prod[:, 128:256])
                nc.vector.tensor_add(sc32[:, 0:64], sc32[:, 0:64], sc32[:, 64:128])
                nc.vector.tensor_add(sc32[:, 0:32], sc32[:, 0:32], sc32[:, 32:64])
                dst = scores[:, t, r0 * H : (r0 + 1) * H]
                nc.vector.tensor_add(dst, sc32[:, 0:16], sc32[:, 16:32])
                for rx in extras:
                    nc.vector.tensor_copy(scores[:, t, rx * H : (rx + 1) * H], dst)

            def emit_softmax(t):
                s_t = scores[:, t, :]
                nc.vector.tensor_mul(s_t, s_t, rw_full[:, t, :])  # *= rel_w
                m = smalls.tile([128, H], F32, tag="m")
                nc.vector.reduce_max(
                    out=m, in_=s_t.rearrange("p (r h) -> p h r", h=H), axis=AX.X
                )
                e_t = p2_all[:, t, :]
                nc.vector.tensor_sub(
                    e_t.rearrange("p (r h) -> p r h", h=H),
                    s_t.rearrange("p (r h) -> p r h", h=H),
                    _bcast_free(m, R, 0),
                )
                # exp((s - m) / sqrt(DH)); the 1/8 rides ACT's free affine
                nc.scalar.activation(e_t, e_t, AF.Exp, scale=0.125)
                ssum = smalls.tile([128, H], F32, tag="ssum")
                nc.vector.reduce_sum(
                    out=ssum, in_=e_t.rearrange("p (r h) -> p h r", h=H), axis=AX.X
                )
                inv = smalls.tile([128, H], F32, tag="inv")
                nc.vector.reciprocal(inv, ssum)
                nc.vector.tensor_mul(
                    e_t.rearrange("p (r h) -> p r h", h=H),
                    e_t.rearrange("p (r h) -> p r h", h=H),
                    _bcast_free(inv, R, 0),
                )
                nc.vector.tensor_mul(e_t, e_t, rw_full[:, t, :])  # P = attn*rel_w
                for j, r0, extras in groups[t]:
                    for rx in extras:
                        nc.vector.tensor_add(
                            e_t[:, r0 * H : (r0 + 1) * H],
                            e_t[:, r0 * H : (r0 + 1) * H],
                            e_t[:, rx * H : (rx + 1) * H],
                        )

            # ---- phase 1: Q+K projections (dense PE stream) + trailing scores ----
            next_pair = 0
            for t in range(T):
                psq0 = psum.tile([128, 512], F32, tag="ps", name="ps")
                psq1 = psum.tile([128, 512], F32, tag="ps", name="ps")
                psk0 = psum.tile([128, 512], F32, tag="ps", name="ps")
                psk1 = psum.tile([128, 512], F32, tag="ps", name="ps")
                for k in range(KCH):
                    lhs = embT[:, k, t * BC : (t + 1) * BC]
                    fl = dict(start=(k == 0), stop=(k == KCH - 1))
                    nc.tensor.matmul(psq0, lhs, wq_t[:, k, 0:512], **fl)
                    nc.tensor.matmul(psq1, lhs, wq_t[:, k, 512:1024], **fl)
                    nc.tensor.matmul(psk0, lhs, wk_t[:, k, 0:512], **fl)
                    nc.tensor.matmul(psk1, lhs, wk_t[:, k, 512:1024], **fl)
                nc.scalar.copy(out=qp_all[:, t, 0:512], in_=psq0)
                nc.scalar.copy(out=qp_all[:, t, 512:1024], in_=psq1)
                nc.scalar.copy(out=kb_all[:, t, 0:512], in_=psk0)
                nc.scalar.copy(out=kb_all[:, t, 512:1024], in_=psk1)
                if use_bq:
                    nc.vector.tensor_add(qp_all[:, t, :], qp_all[:, t, :], bq_full)
                # emit score pairs that just became available
                while next_pair < len(pair_sched) and pair_sched[next_pair][0] <= t:
                    _, tp, jp, r0p, exp_ = pair_sched[next_pair]
                    emit_pair(tp, jp, r0p, exp_)
                    if last_pair_of_t[tp] == next_pair:
                        emit_softmax(tp)
                    next_pair += 1

            wv_t = wpool.tile([128, KCH, D], BF16, tag="w")  # reuses wq slot
            nc.gpsimd.dma_start(
                out=wv_t, in_=wv_ext.rearrange("(k p) o -> p k o", p=128)
            )
            wo_t = wpool.tile([128, KCH, D], BF16, tag="w")  # reuses wk slot
            nc.gpsimd.dma_start(
                out=wo_t, in_=wo_ext.rearrange("(k p) o -> p k o", p=128)
            )

            # ---- phase 2: V projections (recomputed) + ctx accumulation ----
            ctx_all = big.tile([128, T, D], BF16, tag="qpctx")  # reuses qp slot
            done_t = set()
            alt = 0
            for j in range(T):
                psv0 = psum.tile([128, 512], F32, tag="ps", name="ps")
                psv1 = psum.tile([128, 512], F32, tag="ps", name="ps")
                for k in range(KCH):
                    lhs = embT[:, k, j * BC : (j + 1) * BC]
                    fl = dict(start=(k == 0), stop=(k == KCH - 1))
                    nc.tensor.matmul(psv0, lhs, wv_t[:, k, 0:512], **fl)
                    nc.tensor.matmul(psv1, lhs, wv_t[:, k, 512:1024], **fl)
                vb = kvp.tile([128, D], BF16, tag="vb")
                nc.scalar.copy(out=vb[:, 0:512], in_=psv0)
                nc.scalar.copy(out=vb[:, 512:1024], in_=psv1)
                for t, r0 in pairs_by_j[j]:
                    pb = _bcast_free(p2_all[:, t, r0 * H : (r0 + 1) * H], DH, 0)
                    vb3 = vb.rearrange("p (dh h) -> p dh h", h=H)
                    c3 = ctx_all[:, t, :].rearrange("p (dh h) -> p dh h", h=H)
                    if t not in done_t:
                        done_t.add(t)
                        nc.vector.tensor_mul(c3, vb3, pb)
                    else:
                        tmp = work.tile([128, D], BF16, tag="ctmp")
                        nc.vector.tensor_mul(
                            tmp.rearrange("p (dh h) -> p dh h", h=H), vb3, pb
                        )
                        eng = nc.vector if alt % 2 == 0 else nc.gpsimd
                        alt += 1
                        eng.tensor_add(ctx_all[:, t, :], ctx_all[:, t, :], tmp)
                # ship finished ctx rows to DRAM scratch (for xbar re-transpose)
                for t in range(T):
                    if last_j[t] == j:
                        nc.sync.dma_start(out=ctx_dram[t], in_=ctx_all[:, t, :])

            # ---- phase 3: output projection ----
            TG = 2  # t-group size for transposing loads
            for tg in range(T // TG):
                ctxT = ctxTp.tile([128, KCH, TG * BC], BF16)
                for k in range(KCH):
                    nc.scalar.dma_start_transpose(
                        out=ctxT[:, k, :],
                        in_=ctx_dram[
                            tg * TG : (tg + 1) * TG, :, k * 128 : (k + 1) * 128
                        ].rearrange("t b d -> (t b) d"),
                    )
                for ti in range(TG):
                    t = tg * TG + ti
                    o_t = outp.tile([128, D], F32)
                    pso0 = psum.tile([128, 512], F32, tag="ps", name="ps")
                    pso1 = psum.tile([128, 512], F32, tag="ps", name="ps")
                    for k in range(KCH):
                        lhs = ctxT[:, k, ti * BC : (ti + 1) * BC]
                        fl = dict(start=(k == 0), stop=(k == KCH - 1))
                        nc.tensor.matmul(pso0, lhs, wo_t[:, k, 0:512], **fl)
                        nc.tensor.matmul(pso1, lhs, wo_t[:, k, 512:1024], **fl)
                    nc.scalar.copy(out=o_t[:, 0:512], in_=pso0)
                    nc.scalar.copy(out=o_t[:, 512:1024], in_=pso1)
                    if use_bo:
                        nc.vector.tensor_add(o_t, o_t, bo_full)
                    nc.sync.dma_start(out=out_ext[t], in_=o_t)

    return nc


_CACHE = {}


def _get_program(rel_idx, use_bq, use_bo):
    key = (rel_idx.tobytes(), use_bq, use_bo)
    if key not in _CACHE:
        nc = _build(rel_idx, use_bq, use_bo)
        nc.finalize()  # runs the bacc passes (reg alloc, wait lowering, ...)
        _CACHE[key] = nc
    return _CACHE[key]


def kernel(
    table_embs,
    rel_embs,
    rel_idx,
    Wq,
    bq,
    Wk,
    bk,
    Wv,
    bv,
    Wo,
    bo,
    w_rel,
    b_rel,
    _trace=False,
):
    table_embs = np.asarray(table_embs, dtype=np.float32)
    rel_embs = np.asarray(rel_embs, dtype=np.float32)
    rel_idx = np.asarray(rel_idx).astype(np.int64)
    Wq, Wk, Wv, Wo = (np.asarray(w, dtype=np.float32) for w in (Wq, Wk, Wv, Wo))
    bq, bk, bv, bo = (np.asarray(b, dtype=np.float32) for b in (bq, bk, bv, bo))
    w_rel = np.asarray(w_rel, dtype=np.float32)
    b_rel = np.asarray(b_rel, dtype=np.float32)

    # ---- host-side tiny prep ----
    rw = 1.0 / (1.0 + np.exp(-(rel_embs @ w_rel + b_rel[0])))  # [T, R] fp32
    rw_full = np.repeat(rw.astype(np.float32), H, axis=1)  # [T, R*H], col=r*16+h
    bf = ml_dtypes.bfloat16
    wq_p = np.ascontiguousarray(Wq.T[:, _PERM], dtype=bf)
    wk_p = np.ascontiguousarray(Wk.T[:, _PERM], dtype=bf)
    wv_p = np.ascontiguousarray(Wv.T[:, _PERM], dtype=bf)
    wo_p = np.ascontiguousarray(Wo.T[_PERM, :], dtype=bf)
    use_bq = bool(np.any(bq))
    bo_eff = Wo @ bv + bo
    use_bo = bool(np.any(bo_eff))
    bq_p = np.ascontiguousarray(bq[_PERM], dtype=np.float32)

    nc = _get_program(rel_idx, use_bq, use_bo)

    in_maps = []
    for c in range(NCORES):
        m = {
            "emb": np.ascontiguousarray(
                table_embs[:, c * BC : (c + 1) * BC, :], dtype=bf
            ),
            "wq": wq_p,
            "wk": wk_p,
            "wv": wv_p,
            "wo": wo_p,
            "rw": rw_full,
        }
        if use_bq:
            m["bqp"] = bq_p
        if use_bo:
            m["boe"] = bo_eff.astype(np.float32)
        in_maps.append(m)

    res = run_bass_kernel_spmd(nc, in_maps, list(range(NCORES)), trace=_trace)
    out = np.empty((T, B, D), dtype=np.float32)
    for c in range(NCORES):
        out[:, c * BC : (c + 1) * BC, :] = res.results[c]["out"]
    if _trace:
        kernel._last_results = res
    return out



# revision 28
# speedup vs baseline: 1.0048x; 1.0048x over previous
"""CrossTableAttention Trainium2 kernel (8-core SPMD, batch-sharded).

Math (per table t, row b):
  rel_w[t,r]   = sigmoid(rel_embs[t,r] . w_rel + b_rel)          (host, tiny)
  Qp[t]        = emb[t] @ Wq.T (+bq)                              [B, D]
  Kb[j]        = emb[j] @ Wk.T        (bias bk is softmax-invariant -> dropped)
  Vb[j]        = emb[j] @ Wv.T        (bias bv folded into output bias)
  score[t,b,h,r] = rel_w[t,r] * (Qp[t,b,h,:] . Kb[j_r,b,h,:]) / sqrt(DH)
  attn         = softmax_r(score);  P = attn * rel_w
  ctx[t,b]     = sum_r P[t,b,h,r] * Vb[j_r,b,h,:]
  out[t]       = ctx[t] @ Wo.T + (Wo @ bv + bo)

Key algebraic optimization: K/V projections are computed per *table* (16) instead
of per (table, relation) gather (128) - the relation weight is a scalar that
commutes with the linear projection.  5x fewer matmul FLOPs than the reference.

Device layout notes:
 - Activations live as [row(b) on partitions, feature on free] so the attention
   inner products reduce along the free axis (DVE).
 - q/k/v/ctx feature axis is permuted to dh-major (f = dh*16 + h) so the
   per-(b,h) attention-weight broadcast has a step-1 innermost AP dim
   (keeps DVE tensor_tensor in 2x bf16 mode).  Weight matrices are permuted
   host-side to produce/consume this layout directly.
 - scores/P layout per t: [128 b, 128 = r*16 + h] (h innermost).
 - Matmul operands need the contraction dim (d) on partitions; embT is produced
   by bf16 DMA-xbar transposing loads straight from the (host pre-cast) input.
"""

import sys

sys.path.insert(0, "/opt/trn_rl_repo")

import numpy as np
import ml_dtypes

import concourse.bass as bass
import concourse.bacc as bacc_mod
import concourse.mybir as mybir
import concourse.tile as tile
from concourse.bass_utils import run_bass_kernel_spmd

T, B, D, R, H = 16, 1024, 1024, 8, 16
DH = D // H  # 64
NCORES = 8
BC = B // NCORES  # 128 rows per core
KCH = D // 128  # 8 contraction chunks

F32 = mybir.dt.float32
BF16 = mybir.dt.bfloat16
AX = mybir.AxisListType
AF = mybir.ActivationFunctionType

# feature permutation: new f = dh*16 + h  <->  old o = h*64 + dh
_PERM = np.array([(f % H) * DH + f // H for f in range(D)], dtype=np.int64)


def _bcast_free(ap, n, pos):
    """Insert a [step=0, n] broadcast dim into an AP's free dims at `pos`
    (pos counts free dims, 0 = outermost free dim)."""
    new = list(ap.ap)
    new.insert(1 + pos, [0, n])
    return bass.AP(tensor=ap.tensor, offset=ap.offset, ap=new)


def _bcast_part(ap, n=128):
    """Broadcast a (DRAM) AP across n partitions by prepending a [0, n] dim."""
    return bass.AP(tensor=ap.tensor, offset=ap.offset, ap=[[0, n]] + list(ap.ap))


def _structure(rel_idx):
    """Host-side dedup of the (t, j) gather structure."""
    groups = []  # per t: list of (j, r0, [extra r's])
    for t in range(T):
        by_j = {}
        for r in range(R):
            by_j.setdefault(int(rel_idx[t, r]), []).append(r)
        groups.append([(j, rs[0], rs[1:]) for j, rs in sorted(by_j.items())])
    pairs_by_j = {j: [] for j in range(T)}
    for t in range(T):
        for j, r0, extras in groups[t]:
            pairs_by_j[j].append((t, r0))
    # first j (in ascending j processing order) touching each t
    first_j = {}
    for j in range(T):
        for t, _ in pairs_by_j[j]:
            if t not in first_j:
                first_j[t] = j
    # last j touching each t (to schedule ctx output DMA)
    last_j = {}
    for j in range(T):
        for t, _ in pairs_by_j[j]:
            last_j[t] = j
    return groups, pairs_by_j, first_j, last_j


def _build(rel_idx, use_bq, use_bo):
    """Build the SPMD bass program (identical on all cores).

    v2 structure (from v1 trace analysis):
     - Q+K projections interleaved per t (shared stationary embT chunk ->
       LDWEIGHTS amortized over 4 matmuls), Kb fully SBUF-resident so the
       scores phase never back-pressures the PE.
     - score pairs emitted in availability order (sorted by max(t, j)) so the
       DVE trails the projection stream without stalls.
     - per-head dot products via a binary tree of tensor_tensor adds (bf16 2x
       mode for the large levels, fp32 tail) instead of 1x tensor_reduce.
     - V projections recomputed in the ctx phase (keeps PE busy there);
       ctx accumulation adds alternate DVE / GPSIMD.
    """
    groups, pairs_by_j, first_j, last_j = _structure(rel_idx)

    # (t, j, r0, extras) sorted by when both Qp[t] and Kb[j] become available
    pair_sched = []
    for t in range(T):
        for j, r0, extras in groups[t]:
            pair_sched.append((max(t, j), t, j, r0, extras))
    pair_sched.sort(key=lambda x: (x[0], x[1]))
    last_pair_of_t = {}
    for i, (_, t, j, r0, ex) in enumerate(pair_sched):
        last_pair_of_t[t] = i

    nc = bacc_mod.Bacc(None, target_bir_lowering=False, debug=False)
    emb_ext = nc.dram_tensor("emb", [T, BC, D], BF16, kind="ExternalInput")
    wq_ext = nc.dram_tensor("wq", [D, D], BF16, kind="ExternalInput")
    wk_ext = nc.dram_tensor("wk", [D, D], BF16, kind="ExternalInput")
    wv_ext = nc.dram_tensor("wv", [D, D], BF16, kind="ExternalInput")
    wo_ext = nc.dram_tensor("wo", [D, D], BF16, kind="ExternalInput")
    rw_ext = nc.dram_tensor("rw", [T, R * H], F32, kind="ExternalInput")
    if use_bq:
        bq_ext = nc.dram_tensor("bqp", [D], F32, kind="ExternalInput")
    if use_bo:
        bo_ext = nc.dram_tensor("boe", [D], F32, kind="ExternalInput")
    out_ext = nc.dram_tensor("out", [T, BC, D], F32, kind="ExternalOutput")

    with tile.TileContext(nc) as tc:
        with (
            tc.tile_pool(name="consts", bufs=1) as consts,
            tc.tile_pool(name="wpool", bufs=2) as wpool,
            tc.tile_pool(name="big", bufs=1) as big,
            tc.tile_pool(name="embp", bufs=1) as embp,
            tc.tile_pool(name="kball", bufs=1) as kballp,
            tc.tile_pool(name="attn", bufs=1) as attnp,
            tc.tile_pool(name="kv", bufs=3) as kvp,
            tc.tile_pool(name="work", bufs=3) as work,
            tc.tile_pool(name="smalls", bufs=3) as smalls,
            tc.tile_pool(name="outp", bufs=2) as outp,
            tc.tile_pool(name="ctxT", bufs=2) as ctxTp,
            tc.tile_pool(name="psum", bufs=8, space="PSUM") as psum,
            tc.tile_pool(name="dram", bufs=1, space="DRAM") as dramp,
        ):
            ctx_dram = dramp.tile([T, BC, D], BF16)
            # ---- constant + weight loads ----
            rw_full = consts.tile([128, T, R * H], F32)
            nc.gpsimd.dma_start(out=rw_full, in_=_bcast_part(rw_ext[:]))
            if use_bq:
                bq_full = consts.tile([128, D], F32)
                nc.gpsimd.dma_start(out=bq_full, in_=_bcast_part(bq_ext[:]))
            if use_bo:
                bo_full = consts.tile([128, D], F32)
                nc.gpsimd.dma_start(out=bo_full, in_=_bcast_part(bo_ext[:]))

            wq_t = wpool.tile([128, KCH, D], BF16, tag="w")
            nc.gpsimd.dma_start(
                out=wq_t, in_=wq_ext.rearrange("(k p) o -> p k o", p=128)
            )
            wk_t = wpool.tile([128, KCH, D], BF16, tag="w")
            nc.gpsimd.dma_start(
                out=wk_t, in_=wk_ext.rearrange("(k p) o -> p k o", p=128)
            )

            # embT[p, k, t*BC+b] = emb[t, b, k*128+p]  (bf16 xbar transposes,
            # split across both HWDGE sequencers to shorten the startup gate)
            embT = embp.tile([128, KCH, T * BC], BF16)
            for k in range(KCH):
                nc.scalar.dma_start_transpose(
                    out=embT[:, k, :],
                    in_=emb_ext[:, :, k * 128 : (k + 1) * 128].rearrange(
                        "t b d -> (t b) d"
                    ),
                )

            qp_all = big.tile([128, T, D], BF16, tag="qpctx")
            kb_all = kballp.tile([128, T, D], BF16)
            scores = attnp.tile([128, T, R * H], F32)
            p2_all = attnp.tile([128, T, R * H], BF16)

            def emit_pair(t, j, r0, extras):
                """scores[t, r0-block] = per-head dot(Qp[t], Kb[j]) via
                bf16 mul + binary-tree halving over dh (dh-major layout:
                folding dh halves == folding contiguous column halves)."""
                prod = work.tile([128, D], BF16, tag="prod")
                nc.vector.tensor_mul(prod, qp_all[:, t, :], kb_all[:, j, :])
                nc.vector.tensor_add(prod[:, 0:512], prod[:, 0:512], prod[:, 512:1024])
                nc.vector.tensor_add(prod[:, 0:256], prod[:, 0:256], prod[:, 256:512])
                sc32 = smalls.tile([128, 128], F32, tag="sc32")
                nc.vector.tensor_add(sc32, prod[:, 0:128], prod[:, 128:256])
                nc.vector.tensor_add(sc32[:, 0:64], sc32[:, 0:64], sc32[:, 64:128])
                nc.vector.tensor_add(sc32[:, 0:32], sc32[:, 0:32], sc32[:, 32:64])
                dst = scores[:, t, r0 * H : (r0 + 1) * H]
                nc.vector.tensor_add(dst, sc32[:, 0:16], sc32[:, 16:32])
                for rx in extras:
                    nc.vector.tensor_copy(scores[:, t, rx * H : (rx + 1) * H], dst)

            def emit_softmax(t):
                s_t = scores[:, t, :]
                nc.vector.tensor_mul(s_t, s_t, rw_full[:, t, :])  # *= rel_w
                m = smalls.tile([128, H], F32, tag="m")
                nc.vector.reduce_max(
                    out=m, in_=s_t.rearrange("p (r h) -> p h r", h=H), axis=AX.X
                )
                e_t = p2_all[:, t, :]
                nc.vector.tensor_sub(
                    e_t.rearrange("p (r h) -> p r h", h=H),
                    s_t.rearrange("p (r h) -> p r h", h=H),
                    _bcast_free(m, R, 0),
                )
                # exp((s - m) / sqrt(DH)); the 1/8 rides ACT's free affine
                nc.scalar.activation(e_t, e_t, AF.Exp, scale=0.125)
                ssum = smalls.tile([128, H], F32, tag="ssum")
                nc.vector.reduce_sum(
                    out=ssum, in_=e_t.rearrange("p (r h) -> p h r", h=H), axis=AX.X
                )
                inv = smalls.tile([128, H], F32, tag="inv")
                nc.vector.reciprocal(inv, ssum)
                nc.vector.tensor_mul(
                    e_t.rearrange("p (r h) -> p r h", h=H),
                    e_t.rearrange("p (r h) -> p r h", h=H),
                    _bcast_free(inv, R, 0),
                )
                nc.vector.tensor_mul(e_t, e_t, rw_full[:, t, :])  # P = attn*rel_w
                for j, r0, extras in groups[t]:
                    for rx in extras:
                        nc.vector.tensor_add(
                            e_t[:, r0 * H : (r0 + 1) * H],
                            e_t[:, r0 * H : (r0 + 1) * H],
                            e_t[:, rx * H : (rx + 1) * H],
                        )

            # ---- phase 1: Q+K projections (dense PE stream) + trailing scores ----
            next_pair = 0
            for t in range(T):
                psq0 = psum.tile([128, 512], F32, tag="ps", name="ps")
                psq1 = psum.tile([128, 512], F32, tag="ps", name="ps")
                psk0 = psum.tile([128, 512], F32, tag="ps", name="ps")
                psk1 = psum.tile([128, 512], F32, tag="ps", name="ps")
                for k in range(KCH):
                    lhs = embT[:, k, t * BC : (t + 1) * BC]
                    fl = dict(start=(k == 0), stop=(k == KCH - 1))
                    nc.tensor.matmul(psq0, lhs, wq_t[:, k, 0:512], **fl)
                    nc.tensor.matmul(psq1, lhs, wq_t[:, k, 512:1024], **fl)
                    nc.tensor.matmul(psk0, lhs, wk_t[:, k, 0:512], **fl)
                    nc.tensor.matmul(psk1, lhs, wk_t[:, k, 512:1024], **fl)
                nc.scalar.copy(out=qp_all[:, t, 0:512], in_=psq0)
                nc.scalar.copy(out=qp_all[:, t, 512:1024], in_=psq1)
                nc.scalar.copy(out=kb_all[:, t, 0:512], in_=psk0)
                nc.scalar.copy(out=kb_all[:, t, 512:1024], in_=psk1)
                if use_bq:
                    nc.vector.tensor_add(qp_all[:, t, :], qp_all[:, t, :], bq_full)
                # emit score pairs that just became available
                while next_pair < len(pair_sched) and pair_sched[next_pair][0] <= t:
                    _, tp, jp, r0p, exp_ = pair_sched[next_pair]
                    emit_pair(tp, jp, r0p, exp_)
                    if last_pair_of_t[tp] == next_pair:
                        emit_softmax(tp)
                    next_pair += 1

            wv_t = wpool.tile([128, KCH, D], BF16, tag="w")  # reuses wq slot
            nc.gpsimd.dma_start(
                out=wv_t, in_=wv_ext.rearrange("(k p) o -> p k o", p=128)
            )
            wo_t = wpool.tile([128, KCH, D], BF16, tag="w")  # reuses wk slot
            nc.gpsimd.dma_start(
                out=wo_t, in_=wo_ext.rearrange("(k p) o -> p k o", p=128)
            )

            # ---- phase 2: V projections (recomputed) + ctx accumulation ----
            ctx_all = big.tile([128, T, D], BF16, tag="qpctx")  # reuses qp slot
            done_t = set()
            alt = 0
            for j in range(T):
                psv0 = psum.tile([128, 512], F32, tag="ps", name="ps")
                psv1 = psum.tile([128, 512], F32, tag="ps", name="ps")
                for k in range(KCH):
                    lhs = embT[:, k, j * BC : (j + 1) * BC]
                    fl = dict(start=(k == 0), stop=(k == KCH - 1))
                    nc.tensor.matmul(psv0, lhs, wv_t[:, k, 0:512], **fl)
                    nc.tensor.matmul(psv1, lhs, wv_t[:, k, 512:1024], **fl)
                vb = kvp.tile([128, D], BF16, tag="vb")
                nc.scalar.copy(out=vb[:, 0:512], in_=psv0)
                nc.scalar.copy(out=vb[:, 512:1024], in_=psv1)
                for t, r0 in pairs_by_j[j]:
                    pb = _bcast_free(p2_all[:, t, r0 * H : (r0 + 1) * H], DH, 0)
                    vb3 = vb.rearrange("p (dh h) -> p dh h", h=H)
                    c3 = ctx_all[:, t, :].rearrange("p (dh h) -> p dh h", h=H)
                    if t not in done_t:
                        done_t.add(t)
                        nc.vector.tensor_mul(c3, vb3, pb)
                    else:
                        tmp = work.tile([128, D], BF16, tag="ctmp")
                        nc.vector.tensor_mul(
                            tmp.rearrange("p (dh h) -> p dh h", h=H), vb3, pb
                        )
                        eng = nc.vector if alt % 2 == 0 else nc.gpsimd
                        alt += 1
                        eng.tensor_add(ctx_all[:, t, :], ctx_all[:, t, :], tmp)
                # ship finished ctx rows to DRAM scratch (for xbar re-transpose)
                for t in range(T):
                    if last_j[t] == j:
                        nc.sync.dma_start(out=ctx_dram[t], in_=ctx_all[:, t, :])

            # ---- phase 3: output projection ----
            TG = 2  # t-group size for transposing loads
            for tg in range(T // TG):
                ctxT = ctxTp.tile([128, KCH, TG * BC], BF16)
                for k in range(KCH):
                    nc.scalar.dma_start_transpose(
                        out=ctxT[:, k, :],
                        in_=ctx_dram[
                            tg * TG : (tg + 1) * TG, :, k * 128 : (k + 1) * 128
                        ].rearrange("t b d -> (t b) d"),
                    )
                for ti in range(TG):
                    t = tg * TG + ti
                    o_t = outp.tile([128, D], F32)
                    pso0 = psum.tile([128, 512], F32, tag="ps", name="ps")
                    pso1 = psum.tile([128, 512], F32, tag="ps", name="ps")
                    for k in range(KCH):
                        lhs = ctxT[:, k, ti * BC : (ti + 1) * BC]
                        fl = dict(start=(k == 0), stop=(k == KCH - 1))
                        nc.tensor.matmul(pso0, lhs, wo_t[:, k, 0:512], **fl)
                        nc.tensor.matmul(pso1, lhs, wo_t[:, k, 512:1024], **fl)
                    nc.scalar.copy(out=o_t[:, 0:512], in_=pso0)
                    nc.scalar.copy(out=o_t[:, 512:1024], in_=pso1)
                    if use_bo:
                        nc.vector.tensor_add(o_t, o_t, bo_full)
                    nc.sync.dma_start(out=out_ext[t], in_=o_t)

    return nc


_CACHE = {}


def _get_program(rel_idx, use_bq, use_bo):
    key = (rel_idx.tobytes(), use_bq, use_bo)
    if key not in _CACHE:
        nc = _build(rel_idx, use_bq, use_bo)
        nc.finalize()  # runs the bacc passes (reg alloc, wait lowering, ...)
        _CACHE[key] = nc
    return _CACHE[key]


def kernel(
    table_embs,
    rel_embs,
    rel_idx,
    Wq,
    bq,
    Wk,
    bk,
    Wv,
    bv,
    Wo,
    bo,
    w_rel,
    b_rel,
    _trace=False,
):
    table_embs = np.asarray(table_embs, dtype=np.float32)
    rel_embs = np.asarray(rel_embs, dtype=np.float32)
    rel_idx = np.asarray(rel_idx).astype(np.int64)
    Wq, Wk, Wv, Wo = (np.asarray(w, dtype=np.float32) for w in (Wq, Wk, Wv, Wo))
    bq, bk, bv, bo = (np.asarray(b, dtype=np.float32) for b in (bq, bk, bv, bo))
    w_rel = np.asarray(w_rel, dtype=np.float32)
    b_rel = np.asarray(b_rel, dtype=np.float32)

    # ---- host-side tiny prep ----
    rw = 1.0 / (1.0 + np.exp(-(rel_embs @ w_rel + b_rel[0])))  # [T, R] fp32
    rw_full = np.repeat(rw.astype(np.float32), H, axis=1)  # [T, R*H], col=r*16+h
    bf = ml_dtypes.bfloat16
    wq_p = np.ascontiguousarray(Wq.T[:, _PERM], dtype=bf)
    wk_p = np.ascontiguousarray(Wk.T[:, _PERM], dtype=bf)
    wv_p = np.ascontiguousarray(Wv.T[:, _PERM], dtype=bf)
    wo_p = np.ascontiguousarray(Wo.T[_PERM, :], dtype=bf)
    use_bq = bool(np.any(bq))
    bo_eff = Wo @ bv + bo
    use_bo = bool(np.any(bo_eff))
    bq_p = np.ascontiguousarray(bq[_PERM], dtype=np.float32)

    nc = _get_program(rel_idx, use_bq, use_bo)

    in_maps = []
    for c in range(NCORES):
        m = {
            "emb": np.ascontiguousarray(
                table_embs[:, c * BC : (c + 1) * BC, :], dtype=bf
            ),
            "wq": wq_p,
            "wk": wk_p,
            "wv": wv_p,
            "wo": wo_p,
            "rw": rw_full,
        }
        if use_bq:
            m["bqp"] = bq_p
        if use_bo:
            m["boe"] = bo_eff.astype(np.float32)
        in_maps.append(m)

    res = run_bass_kernel_spmd(nc, in_maps, list(range(NCORES)), trace=_trace)
    out = np.empty((T, B, D), dtype=np.float32)
    for c in range(NCORES):
        out[:, c * BC : (c + 1) * BC, :] = res.results[c]["out"]
    if _trace:
        kernel._last_results = res
    return out



# revision 29
# speedup vs baseline: 1.2074x; 1.2016x over previous
"""CrossTableAttention Trainium2 kernel v3 (8-core SPMD, batch-sharded).

Transposed-activation design: every big tensor lives as [feature on
partitions, (table,row) on free].  Projections keep the weights stationary
in the PE and stream all 16 tables' activation columns, the per-head score
reduction runs on the PE via a block-indicator matmul (E), and softmax sits
on 16 partitions (head) with (relation,row) on the free axis so every
partition access starts at 0.  The output is produced transposed and
un-transposed on the host.

Math (per table t, row b):
  rel_w[t,r]  = sigmoid(rel_embs[t,r] . w_rel + b_rel)         (host, tiny)
  qT = Wq'^T embT ; kT = Wk'^T embT ; vT = Wv'^T embT          (PE, stationary W)
  S[t,j][h,b] = sum_d qT[d,tb] kT[d,jb]  (DVE mul -> E-matmul per-head sum)
  e[h,r,b]    = exp(0.125*rel_w[t,r] * S)                      (ACT, AP scale)
  attn        = e / sum_r e  (DVE tree + recip); P = attn*rel_w (gpsimd)
  ctxT[t]    += G-matmul(P rows, dedup r-set) * vT[j]           (PE rep + DVE)
  outT        = Wo'^T ctxT (+ Wo bv + bo)                       (PE, ACT bias)

Feature axis of q/k/v/ctx is dh-major permuted (f = dh*16 + h) so head(f) =
f % 16 = partition % 16 within every 128-chunk: one static indicator E works
for all chunks, and the P->dv replication mask G is shared by all groups.
"""

import sys

sys.path.insert(0, "/opt/trn_rl_repo")

import numpy as np
import ml_dtypes

import concourse.bass as bass
import concourse.bacc as bacc_mod
import concourse.mybir as mybir
import concourse.tile as tile
from concourse.tile_rust import add_dep_helper
from concourse.bass_utils import run_bass_kernel_spmd


def _dep(a, b, why="explicit"):
    add_dep_helper(a.ins, b.ins, reason=why)

T, B, D, R, H = 16, 1024, 1024, 8, 16
DH = D // H
NCORES = 8
BC = B // NCORES  # 128 rows per core
KCH = D // 128  # 8 feature chunks
NS = 4  # tb slices of 512 cols (4 tables each)
TB = T * BC  # 2048

F32 = mybir.dt.float32
BF16 = mybir.dt.bfloat16
AF = mybir.ActivationFunctionType

# feature permutation: new f = dh*16 + h  <->  old o = h*64 + dh
_PERM = np.array([(f % H) * DH + f // H for f in range(D)], dtype=np.int64)
_DEBUG = False


def _bfree(ap, n, pos):
    """Insert a [step=0, n] broadcast free dim at free position `pos`."""
    new = list(ap.ap)
    new.insert(1 + pos, [0, n])
    return bass.AP(tensor=ap.tensor, offset=ap.offset, ap=new)


def _structure(rel_idx):
    """Per t: list of (j, [r's]) dedup groups; plus first/last group info."""
    groups = []
    for t in range(T):
        by_j = {}
        for r in range(R):
            by_j.setdefault(int(rel_idx[t, r]), []).append(r)
        groups.append(sorted(by_j.items()))
    return groups


def _build(rel_idx, use_bq, use_bo):
    groups = _structure(rel_idx)
    # unique pairs (t, j, rs) with readiness stage: q(s) at 2s, k(s) at 2s+1
    pairs = []
    for t in range(T):
        for j, rs in groups[t]:
            stage = max(2 * (t // 4), 2 * (j // 4) + 1)
            pairs.append((stage, t, j, rs))
    pairs.sort(key=lambda x: (x[0], x[1], x[2]))

    nc = bacc_mod.Bacc(None, target_bir_lowering=False, debug=False)
    emb_ext = nc.dram_tensor("emb", [128, KCH, TB], BF16, kind="ExternalInput")
    wq_ext = nc.dram_tensor("wq", [D, D], BF16, kind="ExternalInput")
    wk_ext = nc.dram_tensor("wk", [D, D], BF16, kind="ExternalInput")
    wv_ext = nc.dram_tensor("wv", [D, D], BF16, kind="ExternalInput")
    wo_ext = nc.dram_tensor("wo", [D, D], BF16, kind="ExternalInput")
    ee_ext = nc.dram_tensor("ee", [128, H], BF16, kind="ExternalInput")
    gg_ext = nc.dram_tensor("gg", [H, 128], BF16, kind="ExternalInput")
    rwq_ext = nc.dram_tensor("rwq", [H, T * R], F32, kind="ExternalInput")
    rw2_ext = nc.dram_tensor("rw2", [H, T * R], F32, kind="ExternalInput")
    if use_bq:
        bq_ext = nc.dram_tensor("bqp", [128, KCH], F32, kind="ExternalInput")
    if use_bo:
        bo_ext = nc.dram_tensor("boe", [128, KCH], F32, kind="ExternalInput")
    out_ext = nc.dram_tensor("out", [D, TB], F32, kind="ExternalOutput")
    if _DEBUG:
        qdbg_ext = nc.dram_tensor("qdbg", [128, KCH, TB], BF16, kind="ExternalOutput")
        edbg_ext = nc.dram_tensor("edbg", [H, T, R, BC], BF16, kind="ExternalOutput")

    with tile.TileContext(nc) as tc:
        with (
            tc.tile_pool(name="consts", bufs=1) as consts,
            tc.tile_pool(name="wpool", bufs=2) as wpool,
            tc.tile_pool(name="embp", bufs=1) as embp,
            tc.tile_pool(name="qp", bufs=1) as qpp,
            tc.tile_pool(name="kp", bufs=1) as kpp,
            tc.tile_pool(name="vp", bufs=1) as vpp,
            tc.tile_pool(name="prodp", bufs=2) as prodp,
            tc.tile_pool(name="epp", bufs=1) as epp,
            tc.tile_pool(name="smalls", bufs=1) as smalls,
            tc.tile_pool(name="pbp", bufs=2) as pbp,
            tc.tile_pool(name="outp", bufs=1) as outp,
            tc.tile_pool(name="pps", bufs=3, space="PSUM") as pps,
            tc.tile_pool(name="sps", bufs=2, space="PSUM") as sps,
            tc.tile_pool(name="rps", bufs=2, space="PSUM") as rps,
        ):
            # ---- consts + first weights ----
            E_t = consts.tile([128, H], BF16)
            Ed = nc.gpsimd.dma_start(out=E_t, in_=ee_ext[:])
            G_t = consts.tile([H, 128], BF16)
            Gd = nc.gpsimd.dma_start(out=G_t, in_=gg_ext[:])
            rwq_t = consts.tile([H, T * R], F32)
            rwqd = nc.gpsimd.dma_start(out=rwq_t, in_=rwq_ext[:])
            rw2_t = consts.tile([H, T * R], F32)
            rw2d = nc.gpsimd.dma_start(out=rw2_t, in_=rw2_ext[:])
            if use_bq:
                bq_t = consts.tile([128, KCH], F32)
                nc.gpsimd.dma_start(out=bq_t, in_=bq_ext[:])
            if use_bo:
                bo_t = consts.tile([128, KCH], F32)
                nc.gpsimd.dma_start(out=bo_t, in_=bo_ext[:])
            wq_t = wpool.tile([128, KCH, D], BF16, tag="w", name="w")
            wqd = nc.gpsimd.dma_start(
                out=wq_t, in_=wq_ext.rearrange("(k p) o -> p k o", p=128)
            )
            wk_t = wpool.tile([128, KCH, D], BF16, tag="w", name="w")
            wkd = nc.gpsimd.dma_start(
                out=wk_t, in_=wk_ext.rearrange("(k p) o -> p k o", p=128)
            )
            wdma = {id(wq_t.tensor): wqd, id(wk_t.tensor): wkd}

            # ---- embT: host-pretransposed, plain DMA loads (proven path) ----
            embT = embp.tile([128, KCH, TB], BF16)
            tr_inst = {}
            for kc in range(KCH):
                eng = nc.scalar if kc % 2 == 0 else nc.sync
                ti = eng.dma_start(out=embT[:, kc, :], in_=emb_ext[:, kc, :])
                for s in range(NS):
                    tr_inst[(kc, s)] = ti

            qT = qpp.tile([128, KCH, TB], BF16)
            kT = kpp.tile([128, KCH, TB], BF16, tag="kc", name="kc")
            vT = vpp.tile([128, KCH, TB], BF16)
            eP = epp.tile([H, T, R, BC], BF16)
            ctxTh = [None]  # allocated later from kp pool (reuses kT slot)

            # ---------- emission helpers ----------
            NPAIR_BLK = 2
            blocks = []  # pending dot blocks: list of list[(t, j, rs)]
            cur_blk = []
            exp_done = [0] * T  # r's exp'd per t
            smax_emitted = [False] * T
            ctx_pending = []  # (t, gi, j, rs) in t-major order
            ctx_started = [False] * T
            ctx_done_t = [False] * T
            v_done_s = [False] * NS
            all_dots_emitted = False
            o_emitted = [False] * NS
            smax_inst = {}
            ctx_last = {}

            def emit_block(blk):
                """2-pair dot block: DVE muls + 8 E-matmuls + exp copies."""
                nb = len(blk)
                prod = prodp.tile([128, NPAIR_BLK, KCH, BC], BF16, tag="prod", name="prod")
                pmuls = []
                for i, (t, j, rs) in enumerate(blk):
                    pmuls.append(nc.vector.tensor_mul(
                        prod[:, i, :, :],
                        qT[:, :, t * BC : (t + 1) * BC],
                        kT[:, :, j * BC : (j + 1) * BC],
                    ))
                S = sps.tile([H, NPAIR_BLK * BC], F32, tag="s4", name="s4")
                for kc in range(KCH):
                    mm = nc.tensor.matmul(
                        S[:, 0 : nb * BC],
                        E_t[:],
                        prod[:, 0:nb, kc, :],
                        start=(kc == 0),
                        stop=(kc == KCH - 1),
                    )
                    _dep(mm, Ed, "E dma")
                    for pm in pmuls:
                        _dep(mm, pm, "prod dve write")
                for i, (t, j, rs) in enumerate(blk):
                    for r in rs:
                        ac = nc.scalar.activation(
                            eP[:, t, r, :],
                            S[:, i * BC : (i + 1) * BC],
                            AF.Exp,
                            scale=rwq_t[:, t * R + r : t * R + r + 1],
                        )
                        _dep(ac, rwqd, "rwq dma")
                        exp_done[t] += 1

            def emit_softmax(t):
                """DVE tree-sum over r, recip, normalize; rel_w on gpsimd."""
                sc = smalls.tile([H, 4, BC], BF16, tag="sc", name="sc")
                nc.vector.tensor_add(sc, eP[:, t, 0:4, :], eP[:, t, 4:8, :])
                nc.vector.tensor_add(sc[:, 0:2, :], sc[:, 0:2, :], sc[:, 2:4, :])
                ssum = smalls.tile([H, BC], F32, tag="ssum", name="ssum")
                nc.vector.tensor_add(ssum, sc[:, 0, :], sc[:, 1, :])
                inv = smalls.tile([H, BC], F32, tag="inv", name="inv")
                nc.vector.reciprocal(inv, ssum)
                invb = smalls.tile([H, BC], BF16, tag="invb", name="invb")
                nc.vector.tensor_copy(invb, inv)
                nc.vector.tensor_mul(eP[:, t], eP[:, t], _bfree(invb[:, :], R, 0))
                g = nc.gpsimd.tensor_mul(
                    eP[:, t],
                    eP[:, t],
                    _bfree(rw2_t[:, t * R : (t + 1) * R], BC, 1),
                )
                _dep(g, rw2d, "rw2 dma")
                smax_inst[t] = g

            def emit_ctx(t, j, rs):
                """P replication via G-matmul (dedup-summing the r set), then
                DVE multiply-accumulate into ctxT."""
                prep = rps.tile([128, BC], F32, tag="prep", name="prep")
                for i, r in enumerate(rs):
                    mm = nc.tensor.matmul(
                        prep,
                        G_t[:],
                        eP[:, t, r, :],
                        start=(i == 0),
                        stop=(i == len(rs) - 1),
                    )
                    _dep(mm, Gd, "G dma")
                    _dep(mm, smax_inst[t], "eP final write")
                pb = pbp.tile([128, BC], BF16, tag="pb", name="pb")
                nc.scalar.copy(out=pb, in_=prep)
                dst = ctxTh[0][:, :, t * BC : (t + 1) * BC]
                vsl = vT[:, :, j * BC : (j + 1) * BC]
                if not ctx_started[t]:
                    ctx_started[t] = True
                    ctx_last[t] = nc.vector.tensor_mul(dst, vsl, _bfree(pb[:, :], KCH, 0))
                else:
                    tmp = prodp.tile([128, KCH, BC], BF16, tag="prod", name="ct")
                    nc.vector.tensor_mul(tmp, vsl, _bfree(pb[:, :], KCH, 0))
                    ctx_last[t] = nc.vector.tensor_add(dst, dst, tmp)

            def emit_o_unit(s, oc):
                ps = pps.tile([128, 512], F32, tag="pp", name="pp")
                for kc in range(KCH):
                    mm = nc.tensor.matmul(
                        ps,
                        wo_t[:, kc, oc * 128 : (oc + 1) * 128],
                        ctxTh[0][:, kc, s * 512 : (s + 1) * 512],
                        start=(kc == 0),
                        stop=(kc == KCH - 1),
                    )
                    _dep(mm, wdma[id(wo_t.tensor)], "wo dma")
                    for tt in range(4 * s, 4 * s + 4):
                        _dep(mm, ctx_last[tt], "ctx dve write")
                osb = outp.tile([128, 512], F32, tag="ou", name="ou")
                if use_bo:
                    nc.scalar.activation(
                        osb, ps, AF.Identity, bias=bo_t[:, oc : oc + 1]
                    )
                else:
                    nc.scalar.copy(out=osb, in_=ps)
                nc.sync.dma_start(
                    out=out_ext[oc * 128 : (oc + 1) * 128, s * 512 : (s + 1) * 512],
                    in_=osb,
                )

            def pump(max_blocks=2, max_ctx=3):
                nonlocal blocks
                nb = 0
                while blocks and nb < max_blocks:
                    blk = blocks.pop(0)
                    emit_block(blk)
                    for t in {t for (t, _, _) in blk}:
                        if exp_done[t] == R and not smax_emitted[t]:
                            smax_emitted[t] = True
                            emit_softmax(t)
                    nb += 1
                if not all_dots_emitted:
                    return
                ncx = 0
                while ctx_pending and ncx < max_ctx:
                    t, gi, j, rs = ctx_pending[0]
                    if not (smax_emitted[t] and v_done_s[j // 4]):
                        break
                    ctx_pending.pop(0)
                    emit_ctx(t, j, rs)
                    if gi == len(groups[t]) - 1:
                        ctx_done_t[t] = True
                        s = t // 4
                        if t == 4 * s + 3 and all(
                            ctx_done_t[4 * s : 4 * s + 4]
                        ):
                            o_emitted[s] = True
                            for oc in range(KCH):
                                emit_o_unit(s, oc)
                    ncx += 1

            def queue_pairs(stage):
                nonlocal cur_blk
                for st, t, j, rs in pairs:
                    if st == stage:
                        cur_blk.append((t, j, rs))
                        if len(cur_blk) == NPAIR_BLK:
                            blocks.append(cur_blk)
                            cur_blk = []

            def proj_unit(w_t, dst, s, oc, bias=None):
                ps = pps.tile([128, 512], F32, tag="pp", name="pp")
                for kc in range(KCH):
                    mm = nc.tensor.matmul(
                        ps,
                        w_t[:, kc, oc * 128 : (oc + 1) * 128],
                        embT[:, kc, s * 512 : (s + 1) * 512],
                        start=(kc == 0),
                        stop=(kc == KCH - 1),
                    )
                    _dep(mm, tr_inst[(kc, s)], "embT xbar")
                    _dep(mm, wdma[id(w_t.tensor)], "w dma")
                if bias is not None:
                    nc.scalar.activation(
                        dst[:, oc, s * 512 : (s + 1) * 512],
                        ps,
                        AF.Identity,
                        bias=bias[:, oc : oc + 1],
                    )
                else:
                    nc.scalar.copy(out=dst[:, oc, s * 512 : (s + 1) * 512], in_=ps)

            # ---------- phase 1: Q,K projections (s-major) + trailing dots ----
            for s in range(NS):
                for oc in range(KCH):
                    proj_unit(wq_t, qT, s, oc, bias=bq_t if use_bq else None)
                    pump()
                queue_pairs(2 * s)
                for oc in range(KCH):
                    proj_unit(wk_t, kT, s, oc)
                    pump()
                queue_pairs(2 * s + 1)
            if cur_blk:
                blocks.append(cur_blk)
                cur_blk = []

            # late weights (reuse wq/wk slots once their consumers finish)
            wv_t = wpool.tile([128, KCH, D], BF16, tag="w", name="w")
            wvd = nc.gpsimd.dma_start(
                out=wv_t, in_=wv_ext.rearrange("(k p) o -> p k o", p=128)
            )
            wo_t = wpool.tile([128, KCH, D], BF16, tag="w", name="w")
            wod = nc.gpsimd.dma_start(
                out=wo_t, in_=wo_ext.rearrange("(k p) o -> p k o", p=128)
            )
            wdma[id(wv_t.tensor)] = wvd
            wdma[id(wo_t.tensor)] = wod

            # ---------- phase 2: V projection; dots drain, then ctx ----------
            ctx_queued = False
            for s in range(NS):
                for oc in range(KCH):
                    proj_unit(wv_t, vT, s, oc)
                    if blocks:
                        pump(max_blocks=3, max_ctx=0)
                pump(max_blocks=8, max_ctx=0)
                all_dots_emitted = not blocks
                if all_dots_emitted and not ctx_queued:
                    # ctxT reuses kT's slot (kT dead: all dot muls emitted)
                    ctx_queued = True
                    ctxTh[0] = kpp.tile(
                        [128, KCH, TB], BF16, tag="kc", name="kc"
                    )
                    for t in range(T):
                        for gi, (j, rs) in enumerate(groups[t]):
                            ctx_pending.append((t, gi, j, rs))
                v_done_s[s] = True
                pump(max_blocks=0, max_ctx=8)

            # ---------- phase 3: drain ctx + O ----------
            while ctx_pending:
                before = len(ctx_pending)
                pump(max_blocks=0, max_ctx=1000)
                if len(ctx_pending) == before:
                    raise RuntimeError("ctx scheduling stuck")
            if _DEBUG:
                nc.sync.dma_start(out=qdbg_ext[:], in_=qT)
                nc.sync.dma_start(out=edbg_ext[:], in_=eP)

    return nc


_CACHE = {}


def _get_program(rel_idx, use_bq, use_bo):
    key = (rel_idx.tobytes(), use_bq, use_bo)
    if key not in _CACHE:
        nc = _build(rel_idx, use_bq, use_bo)
        nc.finalize()
        _CACHE[key] = nc
    return _CACHE[key]


def kernel(
    table_embs,
    rel_embs,
    rel_idx,
    Wq,
    bq,
    Wk,
    bk,
    Wv,
    bv,
    Wo,
    bo,
    w_rel,
    b_rel,
    _trace=False,
):
    table_embs = np.asarray(table_embs, dtype=np.float32)
    rel_embs = np.asarray(rel_embs, dtype=np.float32)
    rel_idx = np.asarray(rel_idx).astype(np.int64)
    Wq, Wk, Wv, Wo = (np.asarray(w, dtype=np.float32) for w in (Wq, Wk, Wv, Wo))
    bq, bk, bv, bo = (np.asarray(b, dtype=np.float32) for b in (bq, bk, bv, bo))
    w_rel = np.asarray(w_rel, dtype=np.float32)
    b_rel = np.asarray(b_rel, dtype=np.float32)

    bf = ml_dtypes.bfloat16
    rw = 1.0 / (1.0 + np.exp(-(rel_embs @ w_rel + b_rel[0])))  # [T, R]
    rwq = np.repeat((rw * 0.125).reshape(1, T * R), H, axis=0).astype(np.float32)
    rw2 = np.repeat(rw.reshape(1, T * R), H, axis=0).astype(np.float32)
    E_h = np.zeros((128, H), np.float32)
    for p in range(128):
        E_h[p, p % H] = 1.0
    G_h = np.zeros((H, 128), np.float32)
    for dv in range(128):
        G_h[dv % H, dv] = 1.0

    wq_p = np.ascontiguousarray(Wq.T[:, _PERM], dtype=bf)
    wk_p = np.ascontiguousarray(Wk.T[:, _PERM], dtype=bf)
    wv_p = np.ascontiguousarray(Wv.T[:, _PERM], dtype=bf)
    wo_p = np.ascontiguousarray(Wo.T[_PERM, :], dtype=bf)
    use_bq = bool(np.any(bq))
    bo_eff = Wo @ bv + bo
    use_bo = bool(np.any(bo_eff))
    bq_p = bq[_PERM].reshape(KCH, 128).T.astype(np.float32)  # [128, KCH]
    bo_p = bo_eff.reshape(KCH, 128).T.astype(np.float32)

    nc = _get_program(rel_idx, use_bq, use_bo)

    in_maps = []
    for c in range(NCORES):
        e = np.asarray(
            table_embs[:, c * BC : (c + 1) * BC, :], dtype=bf
        )  # [T, BC, D]
        et = np.ascontiguousarray(
            e.transpose(2, 0, 1).reshape(KCH, 128, TB).transpose(1, 0, 2)
        )  # [128, KCH, TB]: (p, kc, t*BC+b) = emb[t, b, kc*128+p]
        m = {
            "emb": et,
            "wq": wq_p,
            "wk": wk_p,
            "wv": wv_p,
            "wo": wo_p,
            "ee": E_h.astype(bf),
            "gg": G_h.astype(bf),
            "rwq": rwq,
            "rw2": rw2,
        }
        if use_bq:
            m["bqp"] = np.ascontiguousarray(bq_p)
        if use_bo:
            m["boe"] = np.ascontiguousarray(bo_p)
        in_maps.append(m)

    res = run_bass_kernel_spmd(nc, in_maps, list(range(NCORES)), trace=_trace)
    out = np.empty((T, B, D), dtype=np.float32)
    for c in range(NCORES):
        oc = res.results[c]["out"].reshape(D, T, BC)
        out[:, c * BC : (c + 1) * BC, :] = np.transpose(oc, (1, 2, 0))
    kernel._last_results = res
    return out


# revision 30
# speedup vs baseline: 1.4550x; 1.2051x over previous
"""CrossTableAttention Trainium2 kernel v3 (8-core SPMD, batch-sharded).

Transposed-activation design: every big tensor lives as [feature on
partitions, (table,row) on free].  Projections keep the weights stationary
in the PE and stream all 16 tables' activation columns, the per-head score
reduction runs on the PE via a block-indicator matmul (E), and softmax sits
on 16 partitions (head) with (relation,row) on the free axis so every
partition access starts at 0.  The output is produced transposed and
un-transposed on the host.

Math (per table t, row b):
  rel_w[t,r]  = sigmoid(rel_embs[t,r] . w_rel + b_rel)         (host, tiny)
  qT = Wq'^T embT ; kT = Wk'^T embT ; vT = Wv'^T embT          (PE, stationary W)
  S[t,j][h,b] = sum_d qT[d,tb] kT[d,jb]  (DVE mul -> E-matmul per-head sum)
  e[h,r,b]    = exp(0.125*rel_w[t,r] * S)                      (ACT, AP scale)
  attn        = e / sum_r e  (DVE tree + recip); P = attn*rel_w (gpsimd)
  ctxT[t]    += G-matmul(P rows, dedup r-set) * vT[j]           (PE rep + DVE)
  outT        = Wo'^T ctxT (+ Wo bv + bo)                       (PE, ACT bias)

Feature axis of q/k/v/ctx is dh-major permuted (f = dh*16 + h) so head(f) =
f % 16 = partition % 16 within every 128-chunk: one static indicator E works
for all chunks, and the P->dv replication mask G is shared by all groups.
"""

import sys

sys.path.insert(0, "/opt/trn_rl_repo")

import numpy as np
import ml_dtypes

import concourse.bass as bass
import concourse.bacc as bacc_mod
import concourse.mybir as mybir
import concourse.tile as tile
from concourse.tile_rust import add_dep_helper
from concourse.bass_utils import run_bass_kernel_spmd


def _dep(a, b, why="explicit"):
    add_dep_helper(a.ins, b.ins, reason=why)

T, B, D, R, H = 16, 1024, 1024, 8, 16
DH = D // H
NCORES = 8
BC = B // NCORES  # 128 rows per core
KCH = D // 128  # 8 feature chunks
NS = 4  # tb slices of 512 cols (4 tables each)
TB = T * BC  # 2048

F32 = mybir.dt.float32
BF16 = mybir.dt.bfloat16
AF = mybir.ActivationFunctionType

# feature permutation: new f = dh*16 + h  <->  old o = h*64 + dh
_PERM = np.array([(f % H) * DH + f // H for f in range(D)], dtype=np.int64)
_DEBUG = False


def _bfree(ap, n, pos):
    """Insert a [step=0, n] broadcast free dim at free position `pos`."""
    new = list(ap.ap)
    new.insert(1 + pos, [0, n])
    return bass.AP(tensor=ap.tensor, offset=ap.offset, ap=new)


def _structure(rel_idx):
    """Per t: list of (j, [r's]) dedup groups; plus first/last group info."""
    groups = []
    for t in range(T):
        by_j = {}
        for r in range(R):
            by_j.setdefault(int(rel_idx[t, r]), []).append(r)
        groups.append(sorted(by_j.items()))
    return groups


def _build(rel_idx, use_bq, use_bo):
    groups = _structure(rel_idx)
    # unique pairs (t, j, rs) with readiness stage: q(s) at 2s, k(s) at 2s+1
    pairs = []
    for t in range(T):
        for j, rs in groups[t]:
            stage = max(2 * (t // 4), 2 * (j // 4) + 1)
            pairs.append((stage, t, j, rs))
    pairs.sort(key=lambda x: (x[0], x[1], x[2]))

    nc = bacc_mod.Bacc(None, target_bir_lowering=False, debug=False)
    emb_ext = nc.dram_tensor("emb", [128, KCH, TB], BF16, kind="ExternalInput")
    wq_ext = nc.dram_tensor("wq", [D, D], BF16, kind="ExternalInput")
    wk_ext = nc.dram_tensor("wk", [D, D], BF16, kind="ExternalInput")
    wv_ext = nc.dram_tensor("wv", [D, D], BF16, kind="ExternalInput")
    wo_ext = nc.dram_tensor("wo", [D, D], BF16, kind="ExternalInput")
    ee_ext = nc.dram_tensor("ee", [128, H], BF16, kind="ExternalInput")
    gg_ext = nc.dram_tensor("gg", [H, 128], BF16, kind="ExternalInput")
    rwq_ext = nc.dram_tensor("rwq", [H, T * R], F32, kind="ExternalInput")
    rw2_ext = nc.dram_tensor("rw2", [H, T * R], F32, kind="ExternalInput")
    if use_bq:
        bq_ext = nc.dram_tensor("bqp", [128, KCH], F32, kind="ExternalInput")
    if use_bo:
        bo_ext = nc.dram_tensor("boe", [128, KCH], F32, kind="ExternalInput")
    out_ext = nc.dram_tensor("out", [D, TB], F32, kind="ExternalOutput")
    if _DEBUG:
        qdbg_ext = nc.dram_tensor("qdbg", [128, KCH, TB], BF16, kind="ExternalOutput")
        edbg_ext = nc.dram_tensor("edbg", [H, T, R, BC], BF16, kind="ExternalOutput")

    with tile.TileContext(nc) as tc:
        with (
            tc.tile_pool(name="consts", bufs=1) as consts,
            tc.tile_pool(name="wpool", bufs=2) as wpool,
            tc.tile_pool(name="embp", bufs=1) as embp,
            tc.tile_pool(name="qp", bufs=1) as qpp,
            tc.tile_pool(name="kp", bufs=1) as kpp,
            tc.tile_pool(name="vp", bufs=1) as vpp,
            tc.tile_pool(name="prodp", bufs=2) as prodp,
            tc.tile_pool(name="epp", bufs=1) as epp,
            tc.tile_pool(name="smalls", bufs=1) as smalls,
            tc.tile_pool(name="pbp", bufs=4) as pbp,
            tc.tile_pool(name="outp", bufs=1) as outp,
            tc.tile_pool(name="pps", bufs=3, space="PSUM") as pps,
            tc.tile_pool(name="sps", bufs=2, space="PSUM") as sps,
            tc.tile_pool(name="rps", bufs=3, space="PSUM") as rps,
        ):
            # ---- consts + first weights ----
            E_t = consts.tile([128, H], BF16)
            Ed = nc.gpsimd.dma_start(out=E_t, in_=ee_ext[:])
            G_t = consts.tile([H, 128], BF16)
            Gd = nc.gpsimd.dma_start(out=G_t, in_=gg_ext[:])
            rwq_t = consts.tile([H, T * R], F32)
            rwqd = nc.gpsimd.dma_start(out=rwq_t, in_=rwq_ext[:])
            rw2_t = consts.tile([H, T * R], F32)
            rw2d = nc.gpsimd.dma_start(out=rw2_t, in_=rw2_ext[:])
            if use_bq:
                bq_t = consts.tile([128, KCH], F32)
                nc.gpsimd.dma_start(out=bq_t, in_=bq_ext[:])
            if use_bo:
                bo_t = consts.tile([128, KCH], F32)
                nc.gpsimd.dma_start(out=bo_t, in_=bo_ext[:])
            wq_t = wpool.tile([128, KCH, D], BF16, tag="w", name="w")
            wqd = nc.gpsimd.dma_start(
                out=wq_t, in_=wq_ext.rearrange("(k p) o -> p k o", p=128)
            )
            wk_t = wpool.tile([128, KCH, D], BF16, tag="w", name="w")
            wkd = nc.gpsimd.dma_start(
                out=wk_t, in_=wk_ext.rearrange("(k p) o -> p k o", p=128)
            )
            wdma = {id(wq_t.tensor): wqd, id(wk_t.tensor): wkd}

            # ---- embT: host-pretransposed, plain DMA loads (proven path) ----
            embT = embp.tile([128, KCH, TB], BF16)
            tr_inst = {}
            for kc in range(KCH):
                eng = nc.scalar if kc % 2 == 0 else nc.sync
                ti = eng.dma_start(out=embT[:, kc, :], in_=emb_ext[:, kc, :])
                for s in range(NS):
                    tr_inst[(kc, s)] = ti

            qT = qpp.tile([128, KCH, TB], BF16)
            kT = kpp.tile([128, KCH, TB], BF16, tag="kc", name="kc")
            vT = vpp.tile([128, KCH, TB], BF16)
            eP = epp.tile([H, T, R, BC], BF16)
            ctxTh = [None]  # allocated later from kp pool (reuses kT slot)

            # ---------- emission helpers ----------
            NPAIR_BLK = 2
            blocks = []  # pending dot blocks: list of list[(t, j, rs)]
            cur_blk = []
            exp_done = [0] * T  # r's exp'd per t
            smax_emitted = [False] * T
            ctx_pending = []  # (t, gi, j, rs) in t-major order
            ctx_started = [False] * T
            ctx_done_t = [False] * T
            v_done_s = [False] * NS
            all_dots_emitted = False
            o_emitted = [False] * NS
            smax_inst = {}
            ctx_last = {}

            def emit_block(blk):
                """2-pair dot block: DVE muls + 8 E-matmuls + exp copies."""
                nb = len(blk)
                prod = prodp.tile([128, NPAIR_BLK, KCH, BC], BF16, tag="prod", name="prod")
                pmuls = []
                for i, (t, j, rs) in enumerate(blk):
                    pmuls.append(nc.vector.tensor_mul(
                        prod[:, i, :, :],
                        qT[:, :, t * BC : (t + 1) * BC],
                        kT[:, :, j * BC : (j + 1) * BC],
                    ))
                S = sps.tile([H, NPAIR_BLK * BC], F32, tag="s4", name="s4")
                for kc in range(KCH):
                    mm = nc.tensor.matmul(
                        S[:, 0 : nb * BC],
                        E_t[:],
                        prod[:, 0:nb, kc, :],
                        start=(kc == 0),
                        stop=(kc == KCH - 1),
                    )
                    _dep(mm, Ed, "E dma")
                    for pm in pmuls:
                        _dep(mm, pm, "prod dve write")
                for i, (t, j, rs) in enumerate(blk):
                    for r in rs:
                        ac = nc.scalar.activation(
                            eP[:, t, r, :],
                            S[:, i * BC : (i + 1) * BC],
                            AF.Exp,
                            scale=rwq_t[:, t * R + r : t * R + r + 1],
                        )
                        _dep(ac, rwqd, "rwq dma")
                        exp_done[t] += 1

            def emit_softmax(t):
                """DVE tree-sum over r, recip, normalize; rel_w on gpsimd."""
                sc = smalls.tile([H, 4, BC], BF16, tag="sc", name="sc")
                nc.vector.tensor_add(sc, eP[:, t, 0:4, :], eP[:, t, 4:8, :])
                nc.vector.tensor_add(sc[:, 0:2, :], sc[:, 0:2, :], sc[:, 2:4, :])
                ssum = smalls.tile([H, BC], F32, tag="ssum", name="ssum")
                nc.vector.tensor_add(ssum, sc[:, 0, :], sc[:, 1, :])
                inv = smalls.tile([H, BC], F32, tag="inv", name="inv")
                nc.vector.reciprocal(inv, ssum)
                invb = smalls.tile([H, BC], BF16, tag="invb", name="invb")
                nc.vector.tensor_copy(invb, inv)
                nc.vector.tensor_mul(eP[:, t], eP[:, t], _bfree(invb[:, :], R, 0))
                g = nc.gpsimd.tensor_mul(
                    eP[:, t],
                    eP[:, t],
                    _bfree(rw2_t[:, t * R : (t + 1) * R], BC, 1),
                )
                _dep(g, rw2d, "rw2 dma")
                smax_inst[t] = g

            def emit_ctx(t, j, rs):
                """P replication via G-matmul (dedup-summing the r set), then
                DVE multiply-accumulate into ctxT."""
                prep = rps.tile([128, BC], F32, tag="prep", name="prep")
                for i, r in enumerate(rs):
                    mm = nc.tensor.matmul(
                        prep,
                        G_t[:],
                        eP[:, t, r, :],
                        start=(i == 0),
                        stop=(i == len(rs) - 1),
                    )
                    _dep(mm, Gd, "G dma")
                    _dep(mm, smax_inst[t], "eP final write")
                pb = pbp.tile([128, BC], BF16, tag="pb", name="pb")
                nc.scalar.copy(out=pb, in_=prep)
                dst = ctxTh[0][:, :, t * BC : (t + 1) * BC]
                vsl = vT[:, :, j * BC : (j + 1) * BC]
                if not ctx_started[t]:
                    ctx_started[t] = True
                    ctx_last[t] = nc.vector.tensor_mul(dst, vsl, _bfree(pb[:, :], KCH, 0))
                else:
                    tmp = prodp.tile([128, KCH, BC], BF16, tag="prod", name="ct")
                    nc.vector.tensor_mul(tmp, vsl, _bfree(pb[:, :], KCH, 0))
                    ctx_last[t] = nc.vector.tensor_add(dst, dst, tmp)

            def emit_o_unit(s, oc):
                ps = pps.tile([128, 512], F32, tag="pp", name="pp")
                for kc in range(KCH):
                    mm = nc.tensor.matmul(
                        ps,
                        wo_t[:, kc, oc * 128 : (oc + 1) * 128],
                        ctxTh[0][:, kc, s * 512 : (s + 1) * 512],
                        start=(kc == 0),
                        stop=(kc == KCH - 1),
                    )
                    _dep(mm, wdma[id(wo_t.tensor)], "wo dma")
                    for tt in range(4 * s, 4 * s + 4):
                        _dep(mm, ctx_last[tt], "ctx dve write")
                osb = outp.tile([128, 512], F32, tag="ou", name="ou")
                if use_bo:
                    nc.scalar.activation(
                        osb, ps, AF.Identity, bias=bo_t[:, oc : oc + 1]
                    )
                else:
                    nc.scalar.copy(out=osb, in_=ps)
                nc.sync.dma_start(
                    out=out_ext[oc * 128 : (oc + 1) * 128, s * 512 : (s + 1) * 512],
                    in_=osb,
                )

            def pump(max_blocks=2, max_ctx=3):
                nonlocal blocks
                nb = 0
                while blocks and nb < max_blocks:
                    blk = blocks.pop(0)
                    emit_block(blk)
                    for t in {t for (t, _, _) in blk}:
                        if exp_done[t] == R and not smax_emitted[t]:
                            smax_emitted[t] = True
                            emit_softmax(t)
                    nb += 1
                if not all_dots_emitted:
                    return
                ncx = 0
                while ctx_pending and ncx < max_ctx:
                    t, gi, j, rs = ctx_pending[0]
                    if not (smax_emitted[t] and v_done_s[j // 4]):
                        break
                    ctx_pending.pop(0)
                    emit_ctx(t, j, rs)
                    if gi == len(groups[t]) - 1:
                        ctx_done_t[t] = True
                        s = t // 4
                        if t == 4 * s + 3 and all(
                            ctx_done_t[4 * s : 4 * s + 4]
                        ):
                            o_emitted[s] = True
                            for oc in range(KCH):
                                emit_o_unit(s, oc)
                    ncx += 1

            def queue_pairs(stage):
                nonlocal cur_blk
                for st, t, j, rs in pairs:
                    if st == stage:
                        cur_blk.append((t, j, rs))
                        if len(cur_blk) == NPAIR_BLK:
                            blocks.append(cur_blk)
                            cur_blk = []

            def proj_unit(w_t, dst, s, oc, bias=None):
                ps = pps.tile([128, 512], F32, tag="pp", name="pp")
                for kc in range(KCH):
                    mm = nc.tensor.matmul(
                        ps,
                        w_t[:, kc, oc * 128 : (oc + 1) * 128],
                        embT[:, kc, s * 512 : (s + 1) * 512],
                        start=(kc == 0),
                        stop=(kc == KCH - 1),
                    )
                    _dep(mm, tr_inst[(kc, s)], "embT xbar")
                    _dep(mm, wdma[id(w_t.tensor)], "w dma")
                if bias is not None:
                    nc.scalar.activation(
                        dst[:, oc, s * 512 : (s + 1) * 512],
                        ps,
                        AF.Identity,
                        bias=bias[:, oc : oc + 1],
                    )
                else:
                    nc.scalar.copy(out=dst[:, oc, s * 512 : (s + 1) * 512], in_=ps)

            # ---------- phase 1: Q,K projections (s-major) + trailing dots ----
            for s in range(NS):
                for oc in range(KCH):
                    proj_unit(wq_t, qT, s, oc, bias=bq_t if use_bq else None)
                    pump()
                queue_pairs(2 * s)
                for oc in range(KCH):
                    proj_unit(wk_t, kT, s, oc)
                    pump()
                queue_pairs(2 * s + 1)
            if cur_blk:
                blocks.append(cur_blk)
                cur_blk = []

            # late weights (reuse wq/wk slots once their consumers finish)
            wv_t = wpool.tile([128, KCH, D], BF16, tag="w", name="w")
            wvd = nc.gpsimd.dma_start(
                out=wv_t, in_=wv_ext.rearrange("(k p) o -> p k o", p=128)
            )
            wo_t = wpool.tile([128, KCH, D], BF16, tag="w", name="w")
            wod = nc.gpsimd.dma_start(
                out=wo_t, in_=wo_ext.rearrange("(k p) o -> p k o", p=128)
            )
            wdma[id(wv_t.tensor)] = wvd
            wdma[id(wo_t.tensor)] = wod

            # ---------- phase 2: V projection; dots drain, then ctx ----------
            ctx_queued = False
            for s in range(NS):
                for oc in range(KCH):
                    proj_unit(wv_t, vT, s, oc)
                    if blocks:
                        pump(max_blocks=3, max_ctx=0)
                pump(max_blocks=8, max_ctx=0)
                all_dots_emitted = not blocks
                if all_dots_emitted and not ctx_queued:
                    # ctxT reuses kT's slot (kT dead: all dot muls emitted)
                    ctx_queued = True
                    ctxTh[0] = kpp.tile(
                        [128, KCH, TB], BF16, tag="kc", name="kc"
                    )
                    for t in range(T):
                        for gi, (j, rs) in enumerate(groups[t]):
                            ctx_pending.append((t, gi, j, rs))
                v_done_s[s] = True
                pump(max_blocks=0, max_ctx=8)

            # ---------- phase 3: drain ctx + O ----------
            while ctx_pending:
                before = len(ctx_pending)
                pump(max_blocks=0, max_ctx=1000)
                if len(ctx_pending) == before:
                    raise RuntimeError("ctx scheduling stuck")
            if _DEBUG:
                nc.sync.dma_start(out=qdbg_ext[:], in_=qT)
                nc.sync.dma_start(out=edbg_ext[:], in_=eP)

    return nc


_CACHE = {}


def _get_program(rel_idx, use_bq, use_bo):
    key = (rel_idx.tobytes(), use_bq, use_bo)
    if key not in _CACHE:
        nc = _build(rel_idx, use_bq, use_bo)
        nc.finalize()
        _CACHE[key] = nc
    return _CACHE[key]


def kernel(
    table_embs,
    rel_embs,
    rel_idx,
    Wq,
    bq,
    Wk,
    bk,
    Wv,
    bv,
    Wo,
    bo,
    w_rel,
    b_rel,
    _trace=False,
):
    table_embs = np.asarray(table_embs, dtype=np.float32)
    rel_embs = np.asarray(rel_embs, dtype=np.float32)
    rel_idx = np.asarray(rel_idx).astype(np.int64)
    Wq, Wk, Wv, Wo = (np.asarray(w, dtype=np.float32) for w in (Wq, Wk, Wv, Wo))
    bq, bk, bv, bo = (np.asarray(b, dtype=np.float32) for b in (bq, bk, bv, bo))
    w_rel = np.asarray(w_rel, dtype=np.float32)
    b_rel = np.asarray(b_rel, dtype=np.float32)

    bf = ml_dtypes.bfloat16
    rw = 1.0 / (1.0 + np.exp(-(rel_embs @ w_rel + b_rel[0])))  # [T, R]
    rwq = np.repeat((rw * 0.125).reshape(1, T * R), H, axis=0).astype(np.float32)
    rw2 = np.repeat(rw.reshape(1, T * R), H, axis=0).astype(np.float32)
    E_h = np.zeros((128, H), np.float32)
    for p in range(128):
        E_h[p, p % H] = 1.0
    G_h = np.zeros((H, 128), np.float32)
    for dv in range(128):
        G_h[dv % H, dv] = 1.0

    wq_p = np.ascontiguousarray(Wq.T[:, _PERM], dtype=bf)
    wk_p = np.ascontiguousarray(Wk.T[:, _PERM], dtype=bf)
    wv_p = np.ascontiguousarray(Wv.T[:, _PERM], dtype=bf)
    wo_p = np.ascontiguousarray(Wo.T[_PERM, :], dtype=bf)
    use_bq = bool(np.any(bq))
    bo_eff = Wo @ bv + bo
    use_bo = bool(np.any(bo_eff))
    bq_p = bq[_PERM].reshape(KCH, 128).T.astype(np.float32)  # [128, KCH]
    bo_p = bo_eff.reshape(KCH, 128).T.astype(np.float32)

    nc = _get_program(rel_idx, use_bq, use_bo)

    in_maps = []
    for c in range(NCORES):
        e = np.asarray(
            table_embs[:, c * BC : (c + 1) * BC, :], dtype=bf
        )  # [T, BC, D]
        et = np.ascontiguousarray(
            e.transpose(2, 0, 1).reshape(KCH, 128, TB).transpose(1, 0, 2)
        )  # [128, KCH, TB]: (p, kc, t*BC+b) = emb[t, b, kc*128+p]
        m = {
            "emb": et,
            "wq": wq_p,
            "wk": wk_p,
            "wv": wv_p,
            "wo": wo_p,
            "ee": E_h.astype(bf),
            "gg": G_h.astype(bf),
            "rwq": rwq,
            "rw2": rw2,
        }
        if use_bq:
            m["bqp"] = np.ascontiguousarray(bq_p)
        if use_bo:
            m["boe"] = np.ascontiguousarray(bo_p)
        in_maps.append(m)

    res = run_bass_kernel_spmd(nc, in_maps, list(range(NCORES)), trace=_trace)
    out = np.empty((T, B, D), dtype=np.float32)
    for c in range(NCORES):
        oc = res.results[c]["out"].reshape(D, T, BC)
        out[:, c * BC : (c + 1) * BC, :] = np.transpose(oc, (1, 2, 0))
    kernel._last_results = res
    return out
